# revision 8
# baseline (speedup 1.0000x reference)
"""Trainium2 Bass kernel for nn_EulerCausalAttention_75892072121064.

Sharding: batch*heads across 8 cores (core c -> batch c//4, heads 4*(c%4)..+4).
Each core runs an identical program on column-permuted inputs (its 4 heads'
columns moved to the front), computes transposed-layout causal attention for
its (b, 4-head) slice plus the out-proj partial, and writes outT (D, S).
Host sums the 4 per-batch partials and transposes back.

All big matmuls run in float32r (TF32-like, ~1.6e-4 rel err, full PE rate).
The sin/cos LUT of the reference is reproduced exactly: idx = round(theta *
4096/2pi) (f32->i32 cast = round-to-nearest), wrapped to [-2048, 2048] so the
ACT Sin (accurate on [-pi, pi]) evaluates sin/cos at the exact grid angles.
"""
import sys

import numpy as np

sys.path.insert(0, "/opt/trn_rl_repo")

from concourse import bacc, mybir  # noqa: E402
import concourse.tile as tile  # noqa: E402
from concourse.bass_utils import run_bass_kernel_spmd  # noqa: E402

B, S, D, H, DH = 2, 2048, 1024, 16, 64
LUT = 4096
TWO_PI = 2.0 * np.pi
SCALE = float(np.sqrt(np.float32(2.0 * DH)))  # sqrt(128) as f32
NCORES = 8
HPC = 4            # heads per core
CW = HPC * DH      # 256 cols per core
E = 128            # euler feature dim (cos|sin)
SQW = 512          # q window
NQW = S // SQW
KBS = 128          # k block size
C_LUT = float(np.float32(TWO_PI / LUT))

F32 = mybir.dt.float32
F32R = mybir.dt.float32r
I32 = mybir.dt.int32
AF = mybir.ActivationFunctionType
ALU = mybir.AluOpType

_CACHE = {}


def _build_nc():
    nc = bacc.Bacc("TRN2", debug=False, num_devices=NCORES)
    # const AP for the pi/2 Sin bias
    t = nc.alloc_sbuf_tensor("const-f32-halfpi", [128, 1], F32)
    nc.gpsimd.memset(t.ap(), float(np.pi / 2))
    nc.const_aps.aps[(F32, float(np.pi / 2))] = t.ap()
    nc.all_engine_barrier()

    xb = nc.dram_tensor("xb", [S, D], F32, kind="ExternalInput")
    vwT = nc.dram_tensor("vwT", [D, CW], F32, kind="ExternalInput")
    owT = nc.dram_tensor("owT", [CW, D], F32, kind="ExternalInput")
    invq = nc.dram_tensor("invq", [128, 2], F32, kind="ExternalInput")
    bq = nc.dram_tensor("bq", [128, 2], F32, kind="ExternalInput")
    invk = nc.dram_tensor("invk", [128, 2], F32, kind="ExternalInput")
    bk = nc.dram_tensor("bk", [128, 2], F32, kind="ExternalInput")
    tri = nc.dram_tensor("tri", [128, 128], F32, kind="ExternalInput")
    ident = nc.dram_tensor("ident", [128, 128], F32, kind="ExternalInput")
    outT = nc.dram_tensor("outT", [D, S], F32, kind="ExternalOutput")

    NS = S // 128  # number of 128-row seq tiles

    with tile.TileContext(nc) as tc:
        with (
            tc.tile_pool(name="persist", bufs=1) as pp,
            tc.tile_pool(name="qkt", bufs=1) as qkp,
            tc.tile_pool(name="vtiles", bufs=1) as vp,
        ):
            # ---- small constants ----
            ident_sb = pp.tile([128, 128], F32, tag="ident")
            nc.sync.dma_start(ident_sb[:], ident[:])
            tri_sb = pp.tile([128, 128], F32, tag="tri")
            nc.sync.dma_start(tri_sb[:], tri[:])
            invq_sb = pp.tile([128, 2], F32, tag="invq")
            nc.sync.dma_start(invq_sb[:], invq[:])
            bq_sb = pp.tile([128, 2], F32, tag="bq")
            nc.sync.dma_start(bq_sb[:], bq[:])
            invk_sb = pp.tile([128, 2], F32, tag="invk")
            nc.sync.dma_start(invk_sb[:], invk[:])
            bk_sb = pp.tile([128, 2], F32, tag="bk")
            nc.sync.dma_start(bk_sb[:], bk[:])
            ones_r = pp.tile([1, 64], F32R, tag="ones")
            nc.vector.memset(ones_r[:].bitcast(F32), 1.0)

            # owT rows per head, cast to f32r (staging scoped + freed)
            owr = []
            with tc.tile_pool(name="owstage", bufs=2) as ows:
                for h in range(HPC):
                    owf = ows.tile([64, D], F32, tag="owf", name=f"owf{h}")
                    nc.sync.dma_start(owf[:], owT[h * 64:(h + 1) * 64, :])
                    owc = pp.tile([64, D], F32R, tag=f"owr{h}", name=f"owr{h}")
                    nc.vector.tensor_copy(owc[:], owf[:])
                    owr.append(owc)

            # QT/KT assembled feature tiles (f32r), V tiles (f32r, 65-stride)
            qt = [qkp.tile([128, S], F32R, tag=f"qt{h}", name=f"qt{h}") for h in range(HPC)]
            kt = [qkp.tile([128, S], F32R, tag=f"kt{h}", name=f"kt{h}") for h in range(HPC)]
            vt = [vp.tile([128, HPC * 65], F32R, tag=f"v{s}", name=f"v{s}") for s in range(NS)]

            # ---- phase 1a: x -> xT via PE transpose ----
            with (
                tc.tile_pool(name="xT_lo", bufs=1) as xlo,
                tc.tile_pool(name="ph1", bufs=1) as ph1,
            ):
                xT = [None] * 8
                for od in range(2):
                    xT[od] = xlo.tile([128, S], F32, tag=f"xT{od}", name=f"xT{od}")

                with (
                    tc.tile_pool(name="xT_hi", bufs=1) as xhi,
                    tc.tile_pool(name="xstage", bufs=3) as xs_pool,
                    tc.tile_pool(name="tr_ps", bufs=2, space="PSUM") as trp,
                    tc.tile_pool(name="v_ps", bufs=2, space="PSUM") as vps,
                ):
                    for od in range(2, 8):
                        xT[od] = xhi.tile([128, S], F32, tag=f"xT{od}", name=f"xT{od}")

                    # vwT od tiles (fp32; the V matmul runs in plain fp32)
                    vwr = []
                    for od in range(8):
                        vwf = xhi.tile([128, CW], F32, tag=f"vwf{od}", name=f"vwf{od}")
                        nc.sync.dma_start(vwf[:], vwT[od * 128:(od + 1) * 128, :])
                        vwr.append(vwf)

                    for si in range(NS):
                        xs = xs_pool.tile([128, D], F32, tag="xs")
                        nc.sync.dma_start(xs[:], xb[si * 128:(si + 1) * 128, :])
                        for od in range(8):
                            tp = trp.tile([128, 128], F32, tag="tp")
                            nc.tensor.transpose(
                                tp[:], xs[:, od * 128:(od + 1) * 128], ident_sb[:]
                            )
                            eng = nc.vector if od % 2 == 0 else nc.scalar
                            if od % 2 == 0:
                                eng.tensor_copy(
                                    xT[od][:, si * 128:(si + 1) * 128], tp[:]
                                )
                            else:
                                eng.copy(xT[od][:, si * 128:(si + 1) * 128], tp[:])

                    # ---- phase 1b: V = x @ vwT (augmented with ones col) ----
                    for si in range(NS):
                        vpsum = vps.tile([128, CW], F32, tag="vpsum")
                        for od in range(8):
                            nc.tensor.matmul(
                                vpsum[:],
                                xT[od][:, si * 128:(si + 1) * 128],
                                vwr[od][:],
                                start=(od == 0),
                                stop=(od == 7),
                            )
                        # strided copy into [V_h | 1] blocks of width 65
                        dst = vt[si][:].rearrange("p (h w) -> p h w", w=65)[:, :, 0:64]
                        src = vpsum[:].rearrange("p (h w) -> p h w", w=64)
                        nc.vector.tensor_copy(dst, src)
                        onescol = vt[si][:].rearrange("p (h w) -> p h w", w=65)[
                            :, :, 64:65
                        ]
                        nc.gpsimd.memset(onescol.bitcast(F32), 1.0)

                # ---- phase 1c: Euler features from xT[0:2] ----
                FCH = 1024  # feature chunk width
                with tc.tile_pool(name="feat", bufs=1) as fp:
                    for t in range(2):
                        for qk, (inv_sb, b_sb, dstset) in enumerate((
                            (invq_sb, bq_sb, qt),
                            (invk_sb, bk_sb, kt),
                        )):
                            for ch in range(S // FCH):
                                cs = slice(ch * FCH, (ch + 1) * FCH)
                                ts_ = fp.tile([128, FCH], F32, tag="tsmall", name="ts")
                                nc.vector.tensor_scalar(
                                    ts_[:], xT[t][:, cs],
                                    inv_sb[:, t:t + 1], b_sb[:, t:t + 1],
                                    ALU.mult, ALU.add,
                                )
                                ks = fp.tile([128, FCH], F32, tag="ks", name="ks")
                                nc.gpsimd.tensor_scalar(
                                    ks[:], ts_[:], 4096.0, None, ALU.mult
                                )
                                ki = fp.tile([128, FCH], I32, tag="ki", name="ki")
                                nc.vector.tensor_copy(ki[:], ks[:])
                                kf = fp.tile([128, FCH], F32, tag="kf", name="kf")
                                nc.gpsimd.tensor_copy(kf[:], ki[:])
                                # wrap k to [-2048, 2048] without casts (the
                                # f32->i32 rounding mode differs sim vs hw)
                                jp = fp.tile([128, FCH], F32, tag="jp", name="jp")
                                nc.vector.tensor_scalar(
                                    jp[:], kf[:], 2048.0, None, ALU.is_gt
                                )
                                jn = fp.tile([128, FCH], F32, tag="jn", name="jn")
                                nc.vector.tensor_scalar(
                                    jn[:], kf[:], -2048.0, None, ALU.is_lt
                                )
                                kw1 = fp.tile([128, FCH], F32, tag="kw1", name="kw1")
                                nc.vector.scalar_tensor_tensor(
                                    kw1[:], jp[:], -4096.0, kf[:], ALU.mult, ALU.add
                                )
                                kw = fp.tile([128, FCH], F32, tag="kw", name="kw")
                                nc.vector.scalar_tensor_tensor(
                                    kw[:], jn[:], 4096.0, kw1[:], ALU.mult, ALU.add
                                )
                                jc = fp.tile([128, FCH], F32, tag="jc", name="jc")
                                nc.vector.tensor_scalar(
                                    jc[:], kw[:], 1024.0, None, ALU.is_gt
                                )
                                kwc = fp.tile([128, FCH], F32, tag="kwc", name="kwc")
                                nc.vector.scalar_tensor_tensor(
                                    kwc[:], jc[:], -4096.0, kw[:], ALU.mult, ALU.add
                                )
                                # 4 Sin ops -> assembled [cos; sin] tiles
                                for hh in range(2):
                                    dtile = dstset[2 * t + hh]
                                    rows = slice(hh * 64, hh * 64 + 64)
                                    nc.scalar.activation(
                                        dtile[0:64, cs], kwc[rows, :], AF.Sin,
                                        bias=float(np.pi / 2), scale=C_LUT,
                                    )
                                    nc.scalar.activation(
                                        dtile[64:128, cs], kw[rows, :], AF.Sin,
                                        scale=C_LUT,
                                    )

            # ---- phase 2: attention + projection ----
            with (
                tc.tile_pool(name="attnT", bufs=18) as ap,
                tc.tile_pool(name="osb", bufs=2) as op,
                tc.tile_pool(name="sc_ps", bufs=3, space="PSUM") as scp,
                tc.tile_pool(name="o_ps", bufs=2, space="PSUM") as opp,
                tc.tile_pool(name="bc_ps", bufs=1, space="PSUM") as bcp,
                tc.tile_pool(name="pr_ps", bufs=2, space="PSUM") as prp,
            ):
                inv_scale = float(1.0 / np.float32(SCALE))
                for qw in range(NQW):
                    outsb = []
                    for h in range(HPC):
                        nkb = 4 * qw + 4
                        ats = []
                        for kb in range(nkb):
                            sc = scp.tile([128, SQW], F32, tag="sc")
                            nc.tensor.matmul(
                                sc[:],
                                kt[h][:, kb * KBS:(kb + 1) * KBS],
                                qt[h][:, qw * SQW:(qw + 1) * SQW],
                                start=True, stop=True,
                            )
                            at = ap.tile([128, SQW], F32R, tag="attnT")
                            nc.scalar.activation(
                                at[:], sc[:], AF.Exp, scale=inv_scale
                            )
                            r = kb - 4 * qw
                            if r >= 0:
                                nc.vector.tensor_tensor(
                                    at[:, r * 128:(r + 1) * 128],
                                    at[:, r * 128:(r + 1) * 128],
                                    tri_sb[:], ALU.mult,
                                )
                                if r > 0:
                                    nc.gpsimd.memset(at[:, 0:r * 128].bitcast(F32), 0.0)
                            ats.append(at)

                        o_ps = opp.tile([65, SQW], F32, tag="o")
                        for kb in range(nkb):
                            nc.tensor.matmul(
                                o_ps[:],
                                vt[kb][:, h * 65:(h + 1) * 65],
                                ats[kb][:],
                                start=(kb == 0), stop=(kb == nkb - 1),
                            )
                        # normalize: row 64 = sum(exp)
                        srow = op.tile([1, SQW], F32, tag="srow")
                        nc.scalar.copy(srow[:], o_ps[64:65, :])
                        srec = op.tile([1, SQW], F32, tag="srec")
                        nc.vector.reciprocal(srec[:], srow[:])
                        srecr = op.tile([1, SQW], F32R, tag="srecr")
                        nc.gpsimd.tensor_copy(srecr[:], srec[:])
                        bc = bcp.tile([64, SQW], F32, tag="bc")
                        nc.tensor.matmul(
                            bc[:], ones_r[:], srecr[:], start=True, stop=True
                        )
                        oraw = op.tile([64, SQW], F32, tag="oraw")
                        nc.scalar.copy(oraw[:], o_ps[0:64, :])
                        osb = op.tile([64, SQW], F32R, tag=f"osb{h}", bufs=2)
                        nc.vector.tensor_tensor(
                            osb[:], oraw[:], bc[:], ALU.mult
                        )
                        outsb.append(osb)

                    for od in range(8):
                        pr = prp.tile([128, SQW], F32, tag="pr")
                        for h in range(HPC):
                            nc.tensor.matmul(
                                pr[:],
                                owr[h][:, od * 128:(od + 1) * 128],
                                outsb[h][:],
                                start=(h == 0), stop=(h == HPC - 1),
                            )
                        prsb = op.tile([128, SQW], F32, tag="prsb", bufs=4)
                        if od % 2 == 0:
                            nc.vector.tensor_copy(prsb[:], pr[:])
                        else:
                            nc.scalar.copy(prsb[:], pr[:])
                        nc.sync.dma_start(
                            outT[od * 128:(od + 1) * 128,
                                 qw * SQW:(qw + 1) * SQW],
                            prsb[:],
                        )

    nc.compile()
    return nc


def _prep_inputs(x, w_q, b_q, w_k, b_k, v_w, out_w):
    """Build the 8 per-core input maps (host-side sharding)."""
    s_over = np.float64(LUT) / TWO_PI
    in_maps = []
    tri = np.triu(np.ones((128, 128), dtype=np.float32))  # keep q>=k: g>=p
    ident = np.eye(128, dtype=np.float32)

    wq = w_q.reshape(D)
    bqv = b_q.reshape(D)
    wk = w_k.reshape(D)
    bkv = b_k.reshape(D)

    for c in range(NCORES):
        b = c // 4
        h0 = (c % 4) * HPC
        colbase = h0 * DH
        cols = np.arange(colbase, colbase + CW)
        rest = np.concatenate([np.arange(0, colbase), np.arange(colbase + CW, D)])
        perm = np.concatenate([cols, rest])

        xb = np.ascontiguousarray(x[b][:, perm], dtype=np.float32)
        vwT = np.ascontiguousarray(v_w[cols][:, perm].T, dtype=np.float32)
        owT = np.ascontiguousarray(out_w[:, cols].T, dtype=np.float32)

        def featparams(w, bias):
            inv = (s_over / (1.0 + np.abs(w[cols].astype(np.float64))) / LUT)
            bb = bias[cols].astype(np.float64) * s_over / LUT
            return (inv.reshape(2, 128).T.astype(np.float32).copy(),
                    bb.reshape(2, 128).T.astype(np.float32).copy())

        iq, bq_ = featparams(wq, bqv)
        ik, bk_ = featparams(wk, bkv)

        in_maps.append(dict(
            xb=xb, vwT=vwT, owT=owT,
            invq=iq, bq=bq_, invk=ik, bk=bk_,
            tri=tri, ident=ident,
        ))
    return in_maps


def kernel(x, w_q, b_q, w_k, b_k, v_w, out_w, _trace=False):
    x = np.asarray(x, dtype=np.float32)
    w_q = np.asarray(w_q, dtype=np.float32)
    b_q = np.asarray(b_q, dtype=np.float32)
    w_k = np.asarray(w_k, dtype=np.float32)
    b_k = np.asarray(b_k, dtype=np.float32)
    v_w = np.asarray(v_w, dtype=np.float32)
    out_w = np.asarray(out_w, dtype=np.float32)

    if "nc" not in _CACHE:
        _CACHE["nc"] = _build_nc()
    nc = _CACHE["nc"]

    in_maps = _prep_inputs(x, w_q, b_q, w_k, b_k, v_w, out_w)
    res = run_bass_kernel_spmd(
        nc, in_maps, core_ids=list(range(NCORES)), trace=_trace
    )
    out = np.zeros((B, S, D), dtype=np.float32)
    for c in range(NCORES):
        out[c // 4] += res.results[c]["outT"].T
    if _trace:
        kernel._last_result = res
    return out


# revision 12
# speedup vs baseline: 1.5654x; 1.5654x over previous
"""Trainium2 Bass kernel for nn_EulerCausalAttention_75892072121064.

Sharding: batch*heads across 8 cores (core c -> batch c//4, heads 4*(c%4)..+4).
Each core runs an identical program on column-permuted inputs (its 4 heads'
columns moved to the front), computes transposed-layout causal attention for
its (b, 4-head) slice plus the out-proj partial, and writes outT (D, S).
Host sums the 4 per-batch partials and transposes back.

All big matmuls run in float32r (TF32-like, ~1.6e-4 rel err, full PE rate).
The sin/cos LUT of the reference is reproduced exactly: idx = round(theta *
4096/2pi) (f32->i32 cast = round-to-nearest), wrapped to [-2048, 2048] so the
ACT Sin (accurate on [-pi, pi]) evaluates sin/cos at the exact grid angles.
"""
import sys

import numpy as np

sys.path.insert(0, "/opt/trn_rl_repo")

from concourse import bacc, mybir  # noqa: E402
import concourse.tile as tile  # noqa: E402
from concourse.bass_utils import run_bass_kernel_spmd  # noqa: E402

B, S, D, H, DH = 2, 2048, 1024, 16, 64
LUT = 4096
TWO_PI = 2.0 * np.pi
SCALE = float(np.sqrt(np.float32(2.0 * DH)))  # sqrt(128) as f32
NCORES = 8
HPC = 4            # heads per core
CW = HPC * DH      # 256 cols per core
E = 128            # euler feature dim (cos|sin)
SQW = 512          # q window
NQW = S // SQW
KBS = 128          # k block size
C_LUT = float(np.float32(TWO_PI / LUT))

F32 = mybir.dt.float32
F32R = mybir.dt.float32r
I32 = mybir.dt.int32
AF = mybir.ActivationFunctionType
ALU = mybir.AluOpType

_CACHE = {}


def _build_nc():
    nc = bacc.Bacc("TRN2", debug=False, num_devices=NCORES)
    # const AP for the pi/2 Sin bias
    t = nc.alloc_sbuf_tensor("const-f32-halfpi", [128, 1], F32)
    nc.gpsimd.memset(t.ap(), float(np.pi / 2))
    nc.const_aps.aps[(F32, float(np.pi / 2))] = t.ap()
    nc.all_engine_barrier()

    xb = nc.dram_tensor("xb", [S, D], F32, kind="ExternalInput")
    vwT = nc.dram_tensor("vwT", [D, CW], F32, kind="ExternalInput")
    owT = nc.dram_tensor("owT", [CW, D], F32, kind="ExternalInput")
    invq = nc.dram_tensor("invq", [128, 2], F32, kind="ExternalInput")
    bq = nc.dram_tensor("bq", [128, 2], F32, kind="ExternalInput")
    invk = nc.dram_tensor("invk", [128, 2], F32, kind="ExternalInput")
    bk = nc.dram_tensor("bk", [128, 2], F32, kind="ExternalInput")
    tri = nc.dram_tensor("tri", [128, 128], F32, kind="ExternalInput")
    ident = nc.dram_tensor("ident", [128, 128], F32, kind="ExternalInput")
    outT = nc.dram_tensor("outT", [D, S], F32, kind="ExternalOutput")

    NS = S // 128  # number of 128-row seq tiles

    with tile.TileContext(nc) as tc:
        with (
            tc.tile_pool(name="persist", bufs=1) as pp,
            tc.tile_pool(name="qkt", bufs=1) as qkp,
            tc.tile_pool(name="vtiles", bufs=1) as vp,
        ):
            # ---- small constants ----
            ident_sb = pp.tile([128, 128], F32, tag="ident")
            nc.sync.dma_start(ident_sb[:], ident[:])
            tri_sb = pp.tile([128, 128], F32, tag="tri")
            nc.sync.dma_start(tri_sb[:], tri[:])
            invq_sb = pp.tile([128, 2], F32, tag="invq")
            nc.sync.dma_start(invq_sb[:], invq[:])
            bq_sb = pp.tile([128, 2], F32, tag="bq")
            nc.sync.dma_start(bq_sb[:], bq[:])
            invk_sb = pp.tile([128, 2], F32, tag="invk")
            nc.sync.dma_start(invk_sb[:], invk[:])
            bk_sb = pp.tile([128, 2], F32, tag="bk")
            nc.sync.dma_start(bk_sb[:], bk[:])
            ones_r = pp.tile([1, 64], F32R, tag="ones")
            nc.vector.memset(ones_r[:].bitcast(F32), 1.0)

            # owT head-pair tiles (128, D), cast to f32r (staging freed)
            owr = []
            with tc.tile_pool(name="owstage", bufs=2) as ows:
                for hp in range(HPC // 2):
                    owf = ows.tile([128, D], F32, tag="owf", name=f"owf{hp}")
                    nc.sync.dma_start(owf[:], owT[hp * 128:(hp + 1) * 128, :])
                    owc = pp.tile([128, D], F32R, tag=f"owr{hp}", name=f"owr{hp}")
                    nc.vector.tensor_copy(owc[:], owf[:])
                    owr.append(owc)

            # one-hot row-select matrices for the s-recip broadcast matmul
            # (s rows live at partitions 0/32/64/96 - 32-aligned bases)
            sel4 = []
            for h in range(HPC):
                s4 = pp.tile([97, 64], F32R, tag=f"sel{h}", name=f"sel{h}")
                nc.vector.memset(s4[:].bitcast(F32), 0.0)
                nc.vector.memset(s4[32 * h:32 * h + 1, :].bitcast(F32), 1.0)
                sel4.append(s4)

            # QT/KT assembled feature tiles (f32r), V tiles (f32r, 65-stride)
            qt = [qkp.tile([128, S], F32R, tag=f"qt{h}", name=f"qt{h}") for h in range(HPC)]
            kt = [qkp.tile([128, S], F32R, tag=f"kt{h}", name=f"kt{h}") for h in range(HPC)]
            vt = [vp.tile([128, HPC * 65], F32R, tag=f"v{s}", name=f"v{s}") for s in range(NS)]

            # ---- phase 1a: x -> xT via PE transpose ----
            with (
                tc.tile_pool(name="xT_lo", bufs=1) as xlo,
                tc.tile_pool(name="ph1", bufs=1) as ph1,
            ):
                xT = [None] * 8
                for od in range(2):
                    xT[od] = xlo.tile([128, S], F32, tag=f"xT{od}", name=f"xT{od}")

                with (
                    tc.tile_pool(name="xT_hi", bufs=1) as xhi,
                    tc.tile_pool(name="xstage", bufs=3) as xs_pool,
                    tc.tile_pool(name="tr_ps", bufs=2, space="PSUM") as trp,
                    tc.tile_pool(name="v_ps", bufs=2, space="PSUM") as vps,
                ):
                    for od in range(2, 8):
                        xT[od] = xhi.tile([128, S], F32, tag=f"xT{od}", name=f"xT{od}")

                    # vwT od tiles (fp32; the V matmul runs in plain fp32)
                    vwr = []
                    for od in range(8):
                        vwf = xhi.tile([128, CW], F32, tag=f"vwf{od}", name=f"vwf{od}")
                        nc.sync.dma_start(vwf[:], vwT[od * 128:(od + 1) * 128, :])
                        vwr.append(vwf)

                    for si in range(NS):
                        xs = xs_pool.tile([128, D], F32, tag="xs")
                        nc.sync.dma_start(xs[:], xb[si * 128:(si + 1) * 128, :])
                        for od in range(8):
                            tp = trp.tile([128, 128], F32, tag="tp")
                            nc.tensor.transpose(
                                tp[:], xs[:, od * 128:(od + 1) * 128], ident_sb[:]
                            )
                            eng = nc.vector if od % 2 == 0 else nc.scalar
                            if od % 2 == 0:
                                eng.tensor_copy(
                                    xT[od][:, si * 128:(si + 1) * 128], tp[:]
                                )
                            else:
                                eng.copy(xT[od][:, si * 128:(si + 1) * 128], tp[:])

                    # ---- phase 1b: V = x @ vwT (augmented with ones col) ----
                    for si in range(NS):
                        vpsum = vps.tile([128, CW], F32, tag="vpsum")
                        for od in range(8):
                            nc.tensor.matmul(
                                vpsum[:],
                                xT[od][:, si * 128:(si + 1) * 128],
                                vwr[od][:],
                                start=(od == 0),
                                stop=(od == 7),
                            )
                        # strided copy into [V_h | 1] blocks of width 65
                        dst = vt[si][:].rearrange("p (h w) -> p h w", w=65)[:, :, 0:64]
                        src = vpsum[:].rearrange("p (h w) -> p h w", w=64)
                        nc.vector.tensor_copy(dst, src)
                        onescol = vt[si][:].rearrange("p (h w) -> p h w", w=65)[
                            :, :, 64:65
                        ]
                        nc.gpsimd.memset(onescol.bitcast(F32), 1.0)

                # ---- phase 1c: Euler features from xT[0:2] ----
                FCH = 1024  # feature chunk width
                with tc.tile_pool(name="feat", bufs=1) as fp:
                    for t in range(2):
                        for qk, (inv_sb, b_sb, dstset) in enumerate((
                            (invq_sb, bq_sb, qt),
                            (invk_sb, bk_sb, kt),
                        )):
                            for ch in range(S // FCH):
                                cs = slice(ch * FCH, (ch + 1) * FCH)
                                # u = theta*s/4096 (|u| <= ~0.92)
                                ts_ = fp.tile([128, FCH], F32, tag="tsmall", name="ts")
                                nc.vector.tensor_scalar(
                                    ts_[:], xT[t][:, cs],
                                    inv_sb[:, t:t + 1], b_sb[:, t:t + 1],
                                    ALU.mult, ALU.add,
                                )
                                # ks = theta*s (ACT affine copy)
                                ks = fp.tile([128, FCH], F32, tag="ks", name="ks")
                                nc.scalar.activation(ks[:], ts_[:], AF.Copy,
                                                     scale=4096.0)
                                # k = round(ks) via i32 cast (RNE on hw)
                                ki = fp.tile([128, FCH], I32, tag="ki", name="ki")
                                nc.vector.tensor_copy(ki[:], ks[:])
                                kf = fp.tile([128, FCH], F32, tag="kf", name="kf")
                                nc.gpsimd.tensor_copy(kf[:], ki[:])
                                # j = round(u) in {-1,0,1}; kw = k - 4096j
                                ji = fp.tile([128, FCH], I32, tag="ji", name="ji")
                                nc.vector.tensor_copy(ji[:], ts_[:])
                                jf = fp.tile([128, FCH], F32, tag="jf", name="jf")
                                nc.gpsimd.tensor_copy(jf[:], ji[:])
                                kw0 = fp.tile([128, FCH], F32, tag="kw0", name="kw0")
                                nc.vector.scalar_tensor_tensor(
                                    kw0[:], jf[:], -4096.0, kf[:], ALU.mult, ALU.add
                                )
                                # clamp to the Sin domain (identity on hw)
                                kw = fp.tile([128, FCH], F32, tag="kw", name="kw")
                                nc.vector.tensor_scalar(
                                    kw[:], kw0[:], -2048.0, 2048.0, ALU.max, ALU.min
                                )
                                # cos wrap: kwc = kw - 4096*(kw > 1024)
                                jc = fp.tile([128, FCH], F32, tag="jc", name="jc")
                                nc.vector.tensor_scalar(
                                    jc[:], kw[:], 1024.0, -4096.0, ALU.is_gt, ALU.mult
                                )
                                kwc = fp.tile([128, FCH], F32, tag="kwc", name="kwc")
                                nc.vector.tensor_tensor(kwc[:], kw[:], jc[:], ALU.add)
                                # 4 Sin ops -> assembled [cos; sin] tiles
                                for hh in range(2):
                                    dtile = dstset[2 * t + hh]
                                    rows = slice(hh * 64, hh * 64 + 64)
                                    nc.scalar.activation(
                                        dtile[0:64, cs], kwc[rows, :], AF.Sin,
                                        bias=float(np.pi / 2), scale=C_LUT,
                                    )
                                    nc.scalar.activation(
                                        dtile[64:128, cs], kw[rows, :], AF.Sin,
                                        scale=C_LUT,
                                    )

            # ---- phase 2: attention + projection ----
            with (
                tc.tile_pool(name="attnT", bufs=18) as ap,
                tc.tile_pool(name="osb", bufs=2) as op,
                tc.tile_pool(name="sc_ps", bufs=3, space="PSUM") as scp,
                tc.tile_pool(name="o_ps", bufs=2, space="PSUM") as opp,
                tc.tile_pool(name="bc_ps", bufs=1, space="PSUM") as bcp,
                tc.tile_pool(name="pr_ps", bufs=2, space="PSUM") as prp,
            ):
                inv_scale = float(1.0 / np.float32(SCALE))
                for qw in range(NQW):
                    srow4 = op.tile([97, SQW], F32, tag="srow4", name="srow4", bufs=2)
                    nc.gpsimd.memset(srow4[:], 1.0)
                    oraws = []
                    for h in range(HPC):
                        nkb = 4 * qw + 4
                        ats = []
                        for kb in range(nkb):
                            sc = scp.tile([128, SQW], F32, tag="sc", name="sc")
                            nc.tensor.matmul(
                                sc[:],
                                kt[h][:, kb * KBS:(kb + 1) * KBS],
                                qt[h][:, qw * SQW:(qw + 1) * SQW],
                                start=True, stop=True,
                            )
                            at = ap.tile([128, SQW], F32R, tag="attnT", name="at")
                            nc.scalar.activation(
                                at[:], sc[:], AF.Exp, scale=inv_scale
                            )
                            r = kb - 4 * qw
                            if r >= 0:
                                nc.vector.tensor_tensor(
                                    at[:, r * 128:(r + 1) * 128],
                                    at[:, r * 128:(r + 1) * 128],
                                    tri_sb[:], ALU.mult,
                                )
                                if r > 0:
                                    nc.gpsimd.memset(at[:, 0:r * 128].bitcast(F32), 0.0)
                            ats.append(at)

                        o_ps = opp.tile([65, SQW], F32, tag="o", name="o_ps")
                        for kb in range(nkb):
                            nc.tensor.matmul(
                                o_ps[:],
                                vt[kb][:, h * 65:(h + 1) * 65],
                                ats[kb][:],
                                start=(kb == 0), stop=(kb == nkb - 1),
                            )
                        # stash the raw outT and the s row (row 64)
                        nc.scalar.copy(srow4[32 * h:32 * h + 1, :], o_ps[64:65, :])
                        oraw = op.tile([64, SQW], F32, tag=f"oraw{h}",
                                       name=f"oraw{h}", bufs=2)
                        nc.scalar.copy(oraw[:], o_ps[0:64, :])
                        oraws.append(oraw)

                    # batched reciprocal of the 4 softmax denumerator rows
                    srec4 = op.tile([97, SQW], F32, tag="srec4", name="srec4", bufs=2)
                    nc.vector.reciprocal(srec4[:], srow4[:])
                    srecr4 = op.tile([97, SQW], F32R, tag="srecr4", name="srecr4",
                                     bufs=2)
                    nc.vector.tensor_copy(srecr4[:], srec4[:])

                    # normalize into head-pair tiles (128, SQW)
                    pairs = []
                    for hp in range(HPC // 2):
                        pair = op.tile([128, SQW], F32R, tag=f"pair{hp}",
                                       name=f"pair{hp}", bufs=2)
                        pairs.append(pair)
                    for h in range(HPC):
                        bc = bcp.tile([64, SQW], F32, tag="bc", name="bc")
                        nc.tensor.matmul(
                            bc[:], sel4[h][:], srecr4[:], start=True, stop=True
                        )
                        if h % 2 == 0:
                            nc.vector.tensor_tensor(
                                pairs[h // 2][0:64, :], oraws[h][:], bc[:], ALU.mult
                            )
                        else:
                            tmp = op.tile([64, SQW], F32R, tag="ntmp", name="ntmp",
                                          bufs=2)
                            nc.vector.tensor_tensor(
                                tmp[:], oraws[h][:], bc[:], ALU.mult
                            )
                            nc.scalar.copy(
                                pairs[h // 2][64:128, :], tmp[:]
                            )

                    for od in range(8):
                        pr = prp.tile([128, SQW], F32, tag="pr", name="pr")
                        for hp in range(HPC // 2):
                            nc.tensor.matmul(
                                pr[:],
                                owr[hp][:, od * 128:(od + 1) * 128],
                                pairs[hp][:],
                                start=(hp == 0), stop=(hp == HPC // 2 - 1),
                            )
                        prsb = op.tile([128, SQW], F32, tag="prsb", name="prsb",
                                       bufs=4)
                        if od % 2 == 0:
                            nc.vector.tensor_copy(prsb[:], pr[:])
                        else:
                            nc.scalar.copy(prsb[:], pr[:])
                        nc.sync.dma_start(
                            outT[od * 128:(od + 1) * 128,
                                 qw * SQW:(qw + 1) * SQW],
                            prsb[:],
                        )

    nc.compile()
    return nc


def _prep_inputs(x, w_q, b_q, w_k, b_k, v_w, out_w):
    """Build the 8 per-core input maps (host-side sharding)."""
    s_over = np.float64(LUT) / TWO_PI
    in_maps = []
    tri = np.triu(np.ones((128, 128), dtype=np.float32))  # keep q>=k: g>=p
    ident = np.eye(128, dtype=np.float32)

    wq = w_q.reshape(D)
    bqv = b_q.reshape(D)
    wk = w_k.reshape(D)
    bkv = b_k.reshape(D)

    for c in range(NCORES):
        b = c // 4
        h0 = (c % 4) * HPC
        colbase = h0 * DH
        cols = np.arange(colbase, colbase + CW)
        rest = np.concatenate([np.arange(0, colbase), np.arange(colbase + CW, D)])
        perm = np.concatenate([cols, rest])

        xb = np.ascontiguousarray(x[b][:, perm], dtype=np.float32)
        vwT = np.ascontiguousarray(v_w[cols][:, perm].T, dtype=np.float32)
        owT = np.ascontiguousarray(out_w[:, cols].T, dtype=np.float32)

        def featparams(w, bias):
            inv = (s_over / (1.0 + np.abs(w[cols].astype(np.float64))) / LUT)
            bb = bias[cols].astype(np.float64) * s_over / LUT
            return (inv.reshape(2, 128).T.astype(np.float32).copy(),
                    bb.reshape(2, 128).T.astype(np.float32).copy())

        iq, bq_ = featparams(wq, bqv)
        ik, bk_ = featparams(wk, bkv)

        in_maps.append(dict(
            xb=xb, vwT=vwT, owT=owT,
            invq=iq, bq=bq_, invk=ik, bk=bk_,
            tri=tri, ident=ident,
        ))
    return in_maps


def kernel(x, w_q, b_q, w_k, b_k, v_w, out_w, _trace=False):
    x = np.asarray(x, dtype=np.float32)
    w_q = np.asarray(w_q, dtype=np.float32)
    b_q = np.asarray(b_q, dtype=np.float32)
    w_k = np.asarray(w_k, dtype=np.float32)
    b_k = np.asarray(b_k, dtype=np.float32)
    v_w = np.asarray(v_w, dtype=np.float32)
    out_w = np.asarray(out_w, dtype=np.float32)

    if "nc" not in _CACHE:
        _CACHE["nc"] = _build_nc()
    nc = _CACHE["nc"]

    in_maps = _prep_inputs(x, w_q, b_q, w_k, b_k, v_w, out_w)
    res = run_bass_kernel_spmd(
        nc, in_maps, core_ids=list(range(NCORES)), trace=_trace
    )
    out = np.zeros((B, S, D), dtype=np.float32)
    for c in range(NCORES):
        out[c // 4] += res.results[c]["outT"].T
    if _trace:
        kernel._last_result = res
    return out


# revision 13
# speedup vs baseline: 1.8342x; 1.1717x over previous
"""Trainium2 Bass kernel for nn_EulerCausalAttention_75892072121064.

Sharding: batch*heads across 8 cores (core c -> batch c//4, heads 4*(c%4)..+4).
Each core runs an identical program on column-permuted inputs (its 4 heads'
columns moved to the front), computes transposed-layout causal attention for
its (b, 4-head) slice plus the out-proj partial, and writes outT (D, S).
Host sums the 4 per-batch partials and transposes back.

All big matmuls run in float32r (TF32-like, ~1.6e-4 rel err, full PE rate).
The sin/cos LUT of the reference is reproduced exactly: idx = round(theta *
4096/2pi) (f32->i32 cast = round-to-nearest), wrapped to [-2048, 2048] so the
ACT Sin (accurate on [-pi, pi]) evaluates sin/cos at the exact grid angles.
"""
import sys

import numpy as np

sys.path.insert(0, "/opt/trn_rl_repo")

from concourse import bacc, mybir  # noqa: E402
import concourse.tile as tile  # noqa: E402
from concourse.bass_utils import run_bass_kernel_spmd  # noqa: E402

B, S, D, H, DH = 2, 2048, 1024, 16, 64
LUT = 4096
TWO_PI = 2.0 * np.pi
SCALE = float(np.sqrt(np.float32(2.0 * DH)))  # sqrt(128) as f32
NCORES = 8
HPC = 4            # heads per core
CW = HPC * DH      # 256 cols per core
E = 128            # euler feature dim (cos|sin)
SQW = 512          # q window
NQW = S // SQW
KBS = 128          # k block size
C_LUT = float(np.float32(TWO_PI / LUT))

F32 = mybir.dt.float32
F32R = mybir.dt.float32r
I32 = mybir.dt.int32
AF = mybir.ActivationFunctionType
ALU = mybir.AluOpType

_CACHE = {}


def _build_nc():
    nc = bacc.Bacc("TRN2", debug=False, num_devices=NCORES)
    # const AP for the pi/2 Sin bias
    t = nc.alloc_sbuf_tensor("const-f32-halfpi", [128, 1], F32)
    nc.gpsimd.memset(t.ap(), float(np.pi / 2))
    nc.const_aps.aps[(F32, float(np.pi / 2))] = t.ap()
    nc.all_engine_barrier()

    xb = nc.dram_tensor("xb", [S, D], F32, kind="ExternalInput")
    vwT = nc.dram_tensor("vwT", [D, CW], F32, kind="ExternalInput")
    owT = nc.dram_tensor("owT", [CW, D], F32, kind="ExternalInput")
    invq = nc.dram_tensor("invq", [128, 2], F32, kind="ExternalInput")
    bq = nc.dram_tensor("bq", [128, 2], F32, kind="ExternalInput")
    invk = nc.dram_tensor("invk", [128, 2], F32, kind="ExternalInput")
    bk = nc.dram_tensor("bk", [128, 2], F32, kind="ExternalInput")
    tri = nc.dram_tensor("tri", [128, 128], F32, kind="ExternalInput")
    ident = nc.dram_tensor("ident", [128, 128], F32, kind="ExternalInput")
    outT = nc.dram_tensor("outT", [D, S], F32, kind="ExternalOutput")

    NS = S // 128  # number of 128-row seq tiles

    with tile.TileContext(nc) as tc:
        with (
            tc.tile_pool(name="persist", bufs=1) as pp,
            tc.tile_pool(name="qkt", bufs=1) as qkp,
            tc.tile_pool(name="vtiles", bufs=1) as vp,
        ):
            # ---- small constants ----
            ident_sb = pp.tile([128, 128], F32, tag="ident")
            nc.sync.dma_start(ident_sb[:], ident[:])
            tri_sb = pp.tile([128, 128], F32, tag="tri")
            nc.sync.dma_start(tri_sb[:], tri[:])
            invq_sb = pp.tile([128, 2], F32, tag="invq")
            nc.sync.dma_start(invq_sb[:], invq[:])
            bq_sb = pp.tile([128, 2], F32, tag="bq")
            nc.sync.dma_start(bq_sb[:], bq[:])
            invk_sb = pp.tile([128, 2], F32, tag="invk")
            nc.sync.dma_start(invk_sb[:], invk[:])
            bk_sb = pp.tile([128, 2], F32, tag="bk")
            nc.sync.dma_start(bk_sb[:], bk[:])
            ones_r = pp.tile([1, 64], F32R, tag="ones")
            nc.vector.memset(ones_r[:].bitcast(F32), 1.0)

            # one-hot row-select matrices for the s-recip broadcast matmul
            # (s rows live at partitions 0/32/64/96 - 32-aligned bases)
            sel4 = []
            for h in range(HPC):
                s4 = pp.tile([97, 64], F32R, tag=f"sel{h}", name=f"sel{h}")
                nc.vector.memset(s4[:].bitcast(F32), 0.0)
                nc.vector.memset(s4[32 * h:32 * h + 1, :].bitcast(F32), 1.0)
                sel4.append(s4)

            # QT/KT assembled feature tiles (f32r), V tiles (f32r, 65-stride)
            qt = [qkp.tile([128, S], F32R, tag=f"qt{h}", name=f"qt{h}") for h in range(HPC)]
            kt = [qkp.tile([128, S], F32R, tag=f"kt{h}", name=f"kt{h}") for h in range(HPC)]
            vt = [vp.tile([128, HPC * 65], F32R, tag=f"v{s}", name=f"v{s}") for s in range(NS)]

            # ---- phase 1: transpose passes + features + V ----
            with (
                tc.tile_pool(name="xT_lo", bufs=1) as xlo,
                tc.tile_pool(name="ph1", bufs=1) as ph1,
            ):
                xT = [None] * 8
                for od in range(2):
                    xT[od] = xlo.tile([128, S], F32R, tag=f"xT{od}", name=f"xT{od}")

                with (
                    tc.tile_pool(name="xT_hi", bufs=1) as xhi,
                    tc.tile_pool(name="xstage", bufs=2) as xs_pool,
                    tc.tile_pool(name="vwst", bufs=2) as vwst,
                    tc.tile_pool(name="tr_ps", bufs=2, space="PSUM") as trp,
                    tc.tile_pool(name="v_ps", bufs=2, space="PSUM") as vps,
                ):
                    for od in range(2, 8):
                        xT[od] = xhi.tile([128, S], F32R, tag=f"xT{od}", name=f"xT{od}")

                    # vwT od tiles -> f32r
                    vwr = []
                    for od in range(8):
                        vwf = vwst.tile([128, CW], F32, tag="vwf", name=f"vwf{od}")
                        nc.sync.dma_start(vwf[:], vwT[od * 128:(od + 1) * 128, :])
                        vwc = ph1.tile([128, CW], F32R, tag=f"vwr{od}", name=f"vwr{od}")
                        nc.vector.tensor_copy(vwc[:], vwf[:])
                        vwr.append(vwc)

                    def transpose_od(si, xs, od):
                        tp = trp.tile([128, 128], F32, tag="tp", name="tp")
                        nc.tensor.transpose(
                            tp[:], xs[:, od * 128:(od + 1) * 128], ident_sb[:]
                        )
                        if od % 2 == 0:
                            nc.vector.tensor_copy(
                                xT[od][:, si * 128:(si + 1) * 128], tp[:]
                            )
                        else:
                            nc.scalar.copy(xT[od][:, si * 128:(si + 1) * 128], tp[:])

                    # pass A: od 0,1 only (unblocks features early)
                    for si in range(NS):
                        xs = xs_pool.tile([128, D], F32, tag="xs", name="xsA")
                        nc.sync.dma_start(xs[:], xb[si * 128:(si + 1) * 128, :])
                        for od in range(2):
                            transpose_od(si, xs, od)

                    # ---- Euler features from xT[0:2] ----
                    FCH = 512  # feature chunk width
                    with tc.tile_pool(name="feat", bufs=1) as fp:
                        for t in range(2):
                            for qk, (inv_sb, b_sb, dstset) in enumerate((
                                (invq_sb, bq_sb, qt),
                                (invk_sb, bk_sb, kt),
                            )):
                                for ch in range(S // FCH):
                                    cs = slice(ch * FCH, (ch + 1) * FCH)
                                    # u = theta*s/4096 (|u| <= ~0.92)
                                    ts_ = fp.tile([128, FCH], F32, tag="tA", name="ts")
                                    nc.vector.tensor_scalar(
                                        ts_[:], xT[t][:, cs],
                                        inv_sb[:, t:t + 1], b_sb[:, t:t + 1],
                                        ALU.mult, ALU.add,
                                    )
                                    # ks = theta*s (ACT affine copy)
                                    ks = fp.tile([128, FCH], F32, tag="tB", name="ks")
                                    nc.scalar.activation(ks[:], ts_[:], AF.Copy,
                                                         scale=4096.0)
                                    # k = round(ks) via i32 cast (RNE on hw)
                                    ki = fp.tile([128, FCH], I32, tag="tC", name="ki")
                                    nc.vector.tensor_copy(ki[:], ks[:])
                                    ji = fp.tile([128, FCH], I32, tag="tD", name="ji")
                                    nc.vector.tensor_copy(ji[:], ts_[:])
                                    kf = fp.tile([128, FCH], F32, tag="tB", name="kf")
                                    nc.vector.tensor_copy(kf[:], ki[:])
                                    jf = fp.tile([128, FCH], F32, tag="tA", name="jf")
                                    nc.vector.tensor_copy(jf[:], ji[:])
                                    kw0 = fp.tile([128, FCH], F32, tag="tE", name="kw0")
                                    nc.vector.scalar_tensor_tensor(
                                        kw0[:], jf[:], -4096.0, kf[:], ALU.mult, ALU.add
                                    )
                                    # clamp to the Sin domain (identity on hw)
                                    kw = fp.tile([128, FCH], F32, tag="tF", name="kw")
                                    nc.vector.tensor_scalar(
                                        kw[:], kw0[:], -2048.0, 2048.0, ALU.max, ALU.min
                                    )
                                    # cos wrap: kwc = kw - 4096*(kw > 1024)
                                    jc = fp.tile([128, FCH], F32, tag="tE", name="jc")
                                    nc.vector.tensor_scalar(
                                        jc[:], kw[:], 1024.0, -4096.0, ALU.is_gt,
                                        ALU.mult
                                    )
                                    kwc = fp.tile([128, FCH], F32, tag="tD", name="kwc")
                                    nc.vector.tensor_tensor(kwc[:], kw[:], jc[:],
                                                            ALU.add)
                                    # 4 Sin ops -> assembled [cos; sin] tiles
                                    for hh in range(2):
                                        dtile = dstset[2 * t + hh]
                                        rows = slice(hh * 64, hh * 64 + 64)
                                        nc.scalar.activation(
                                            dtile[0:64, cs], kwc[rows, :], AF.Sin,
                                            bias=float(np.pi / 2), scale=C_LUT,
                                        )
                                        nc.scalar.activation(
                                            dtile[64:128, cs], kw[rows, :], AF.Sin,
                                            scale=C_LUT,
                                        )

                    # pass B: od 2..7 (re-DMA x)
                    for si in range(NS):
                        xs = xs_pool.tile([128, D], F32, tag="xs", name="xsB")
                        nc.sync.dma_start(xs[:], xb[si * 128:(si + 1) * 128, :])
                        for od in range(2, 8):
                            transpose_od(si, xs, od)

                    # ---- V = x @ vwT (f32r, augmented with ones col) ----
                    for si in range(NS):
                        vpsum = vps.tile([128, CW], F32, tag="vpsum", name="vpsum")
                        for od in range(8):
                            nc.tensor.matmul(
                                vpsum[:],
                                xT[od][:, si * 128:(si + 1) * 128],
                                vwr[od][:],
                                start=(od == 0),
                                stop=(od == 7),
                            )
                        # strided copy into [V_h | 1] blocks of width 65
                        dst = vt[si][:].rearrange("p (h w) -> p h w", w=65)[:, :, 0:64]
                        src = vpsum[:].rearrange("p (h w) -> p h w", w=64)
                        nc.vector.tensor_copy(dst, src)
                        onescol = vt[si][:].rearrange("p (h w) -> p h w", w=65)[
                            :, :, 64:65
                        ]
                        nc.gpsimd.memset(onescol.bitcast(F32), 1.0)

            # owT head-pair tiles (128, D), cast to f32r (staging freed)
            owr = []
            with tc.tile_pool(name="owstage", bufs=2) as ows:
                for hp in range(HPC // 2):
                    owf = ows.tile([128, D], F32, tag="owf", name=f"owf{hp}")
                    nc.sync.dma_start(owf[:], owT[hp * 128:(hp + 1) * 128, :])
                    owc = pp.tile([128, D], F32R, tag=f"owr{hp}", name=f"owr{hp}")
                    nc.vector.tensor_copy(owc[:], owf[:])
                    owr.append(owc)

            # ---- phase 2: attention + projection ----
            with (
                tc.tile_pool(name="attnT", bufs=18) as ap,
                tc.tile_pool(name="osb", bufs=2) as op,
                tc.tile_pool(name="sc_ps", bufs=3, space="PSUM") as scp,
                tc.tile_pool(name="o_ps", bufs=2, space="PSUM") as opp,
                tc.tile_pool(name="bc_ps", bufs=1, space="PSUM") as bcp,
                tc.tile_pool(name="pr_ps", bufs=2, space="PSUM") as prp,
            ):
                inv_scale = float(1.0 / np.float32(SCALE))
                for qw in range(NQW):
                    srow4 = op.tile([97, SQW], F32, tag="srow4", name="srow4", bufs=2)
                    nc.gpsimd.memset(srow4[:], 1.0)
                    oraws = []
                    for h in range(HPC):
                        nkb = 4 * qw + 4
                        ats = []
                        for kb in range(nkb):
                            sc = scp.tile([128, SQW], F32, tag="sc", name="sc")
                            nc.tensor.matmul(
                                sc[:],
                                kt[h][:, kb * KBS:(kb + 1) * KBS],
                                qt[h][:, qw * SQW:(qw + 1) * SQW],
                                start=True, stop=True,
                            )
                            at = ap.tile([128, SQW], F32R, tag="attnT", name="at")
                            nc.scalar.activation(
                                at[:], sc[:], AF.Exp, scale=inv_scale
                            )
                            r = kb - 4 * qw
                            if r >= 0:
                                nc.vector.tensor_tensor(
                                    at[:, r * 128:(r + 1) * 128],
                                    at[:, r * 128:(r + 1) * 128],
                                    tri_sb[:], ALU.mult,
                                )
                                if r > 0:
                                    nc.gpsimd.memset(at[:, 0:r * 128].bitcast(F32), 0.0)
                            ats.append(at)

                        o_ps = opp.tile([65, SQW], F32, tag="o", name="o_ps")
                        for kb in range(nkb):
                            nc.tensor.matmul(
                                o_ps[:],
                                vt[kb][:, h * 65:(h + 1) * 65],
                                ats[kb][:],
                                start=(kb == 0), stop=(kb == nkb - 1),
                            )
                        # stash the raw outT and the s row (row 64)
                        nc.scalar.copy(srow4[32 * h:32 * h + 1, :], o_ps[64:65, :])
                        oraw = op.tile([64, SQW], F32, tag=f"oraw{h}",
                                       name=f"oraw{h}", bufs=2)
                        nc.scalar.copy(oraw[:], o_ps[0:64, :])
                        oraws.append(oraw)

                    # batched reciprocal of the 4 softmax denumerator rows
                    srec4 = op.tile([97, SQW], F32, tag="srec4", name="srec4", bufs=2)
                    nc.vector.reciprocal(srec4[:], srow4[:])
                    srecr4 = op.tile([97, SQW], F32R, tag="srecr4", name="srecr4",
                                     bufs=2)
                    nc.vector.tensor_copy(srecr4[:], srec4[:])

                    # normalize into head-pair tiles (128, SQW)
                    pairs = []
                    for hp in range(HPC // 2):
                        pair = op.tile([128, SQW], F32R, tag=f"pair{hp}",
                                       name=f"pair{hp}", bufs=2)
                        pairs.append(pair)
                    for h in range(HPC):
                        bc = bcp.tile([64, SQW], F32, tag="bc", name="bc")
                        nc.tensor.matmul(
                            bc[:], sel4[h][:], srecr4[:], start=True, stop=True
                        )
                        if h % 2 == 0:
                            nc.vector.tensor_tensor(
                                pairs[h // 2][0:64, :], oraws[h][:], bc[:], ALU.mult
                            )
                        else:
                            tmp = op.tile([64, SQW], F32R, tag="ntmp", name="ntmp",
                                          bufs=2)
                            nc.vector.tensor_tensor(
                                tmp[:], oraws[h][:], bc[:], ALU.mult
                            )
                            nc.scalar.copy(
                                pairs[h // 2][64:128, :], tmp[:]
                            )

                    for od in range(8):
                        pr = prp.tile([128, SQW], F32, tag="pr", name="pr")
                        for hp in range(HPC // 2):
                            nc.tensor.matmul(
                                pr[:],
                                owr[hp][:, od * 128:(od + 1) * 128],
                                pairs[hp][:],
                                start=(hp == 0), stop=(hp == HPC // 2 - 1),
                            )
                        prsb = op.tile([128, SQW], F32, tag="prsb", name="prsb",
                                       bufs=4)
                        if od % 2 == 0:
                            nc.vector.tensor_copy(prsb[:], pr[:])
                        else:
                            nc.scalar.copy(prsb[:], pr[:])
                        nc.sync.dma_start(
                            outT[od * 128:(od + 1) * 128,
                                 qw * SQW:(qw + 1) * SQW],
                            prsb[:],
                        )

    nc.compile()
    return nc


def _prep_inputs(x, w_q, b_q, w_k, b_k, v_w, out_w):
    """Build the 8 per-core input maps (host-side sharding)."""
    s_over = np.float64(LUT) / TWO_PI
    in_maps = []
    tri = np.triu(np.ones((128, 128), dtype=np.float32))  # keep q>=k: g>=p
    ident = np.eye(128, dtype=np.float32)

    wq = w_q.reshape(D)
    bqv = b_q.reshape(D)
    wk = w_k.reshape(D)
    bkv = b_k.reshape(D)

    for c in range(NCORES):
        b = c // 4
        h0 = (c % 4) * HPC
        colbase = h0 * DH
        cols = np.arange(colbase, colbase + CW)
        rest = np.concatenate([np.arange(0, colbase), np.arange(colbase + CW, D)])
        perm = np.concatenate([cols, rest])

        xb = np.ascontiguousarray(x[b][:, perm], dtype=np.float32)
        vwT = np.ascontiguousarray(v_w[cols][:, perm].T, dtype=np.float32)
        owT = np.ascontiguousarray(out_w[:, cols].T, dtype=np.float32)

        def featparams(w, bias):
            inv = (s_over / (1.0 + np.abs(w[cols].astype(np.float64))) / LUT)
            bb = bias[cols].astype(np.float64) * s_over / LUT
            return (inv.reshape(2, 128).T.astype(np.float32).copy(),
                    bb.reshape(2, 128).T.astype(np.float32).copy())

        iq, bq_ = featparams(wq, bqv)
        ik, bk_ = featparams(wk, bkv)

        in_maps.append(dict(
            xb=xb, vwT=vwT, owT=owT,
            invq=iq, bq=bq_, invk=ik, bk=bk_,
            tri=tri, ident=ident,
        ))
    return in_maps


def kernel(x, w_q, b_q, w_k, b_k, v_w, out_w, _trace=False):
    x = np.asarray(x, dtype=np.float32)
    w_q = np.asarray(w_q, dtype=np.float32)
    b_q = np.asarray(b_q, dtype=np.float32)
    w_k = np.asarray(w_k, dtype=np.float32)
    b_k = np.asarray(b_k, dtype=np.float32)
    v_w = np.asarray(v_w, dtype=np.float32)
    out_w = np.asarray(out_w, dtype=np.float32)

    if "nc" not in _CACHE:
        _CACHE["nc"] = _build_nc()
    nc = _CACHE["nc"]

    in_maps = _prep_inputs(x, w_q, b_q, w_k, b_k, v_w, out_w)
    res = run_bass_kernel_spmd(
        nc, in_maps, core_ids=list(range(NCORES)), trace=_trace
    )
    out = np.zeros((B, S, D), dtype=np.float32)
    for c in range(NCORES):
        out[c // 4] += res.results[c]["outT"].T
    if _trace:
        kernel._last_result = res
    return out


# revision 14
# speedup vs baseline: 1.9578x; 1.0674x over previous
"""Trainium2 Bass kernel for nn_EulerCausalAttention_75892072121064.

Sharding: batch*heads across 8 cores (core c -> batch c//4, heads 4*(c%4)..+4).
Each core runs an identical program on column-permuted inputs (its 4 heads'
columns moved to the front), computes transposed-layout causal attention for
its (b, 4-head) slice plus the out-proj partial, and writes outT (D, S).
Host sums the 4 per-batch partials and transposes back.

All big matmuls run in float32r (TF32-like, ~1.6e-4 rel err, full PE rate).
The sin/cos LUT of the reference is reproduced exactly: idx = round(theta *
4096/2pi) (f32->i32 cast = round-to-nearest), wrapped to [-2048, 2048] so the
ACT Sin (accurate on [-pi, pi]) evaluates sin/cos at the exact grid angles.
"""
import sys

import numpy as np

sys.path.insert(0, "/opt/trn_rl_repo")

from concourse import bacc, mybir  # noqa: E402
import concourse.tile as tile  # noqa: E402
from concourse.bass_utils import run_bass_kernel_spmd  # noqa: E402

B, S, D, H, DH = 2, 2048, 1024, 16, 64
LUT = 4096
TWO_PI = 2.0 * np.pi
SCALE = float(np.sqrt(np.float32(2.0 * DH)))  # sqrt(128) as f32
NCORES = 8
HPC = 4            # heads per core
CW = HPC * DH      # 256 cols per core
E = 128            # euler feature dim (cos|sin)
SQW = 512          # q window
NQW = S // SQW
KBS = 128          # k block size
C_LUT = float(np.float32(TWO_PI / LUT))

F32 = mybir.dt.float32
F32R = mybir.dt.float32r
I32 = mybir.dt.int32
AF = mybir.ActivationFunctionType
ALU = mybir.AluOpType

_CACHE = {}


def _build_nc():
    nc = bacc.Bacc("TRN2", debug=False, num_devices=NCORES)
    # const AP for the pi/2 Sin bias
    t = nc.alloc_sbuf_tensor("const-f32-halfpi", [128, 1], F32)
    nc.gpsimd.memset(t.ap(), float(np.pi / 2))
    nc.const_aps.aps[(F32, float(np.pi / 2))] = t.ap()
    nc.all_engine_barrier()

    xb = nc.dram_tensor("xb", [S, D], F32, kind="ExternalInput")
    vwT = nc.dram_tensor("vwT", [D, CW], F32, kind="ExternalInput")
    owT = nc.dram_tensor("owT", [CW, D], F32, kind="ExternalInput")
    invq = nc.dram_tensor("invq", [128, 2], F32, kind="ExternalInput")
    bq = nc.dram_tensor("bq", [128, 2], F32, kind="ExternalInput")
    invk = nc.dram_tensor("invk", [128, 2], F32, kind="ExternalInput")
    bk = nc.dram_tensor("bk", [128, 2], F32, kind="ExternalInput")
    tri = nc.dram_tensor("tri", [128, 128], F32, kind="ExternalInput")
    ident = nc.dram_tensor("ident", [128, 128], F32, kind="ExternalInput")
    outT = nc.dram_tensor("outT", [D, S], F32, kind="ExternalOutput")

    NS = S // 128  # number of 128-row seq tiles

    with tile.TileContext(nc) as tc:
        with (
            tc.tile_pool(name="persist", bufs=1) as pp,
            tc.tile_pool(name="qkt", bufs=1) as qkp,
            tc.tile_pool(name="vtiles", bufs=1) as vp,
        ):
            # ---- small constants ----
            ident_sb = pp.tile([128, 128], F32, tag="ident")
            nc.sync.dma_start(ident_sb[:], ident[:])
            tri_sb = pp.tile([128, 128], F32, tag="tri")
            nc.sync.dma_start(tri_sb[:], tri[:])
            invq_sb = pp.tile([128, 2], F32, tag="invq")
            nc.sync.dma_start(invq_sb[:], invq[:])
            bq_sb = pp.tile([128, 2], F32, tag="bq")
            nc.sync.dma_start(bq_sb[:], bq[:])
            invk_sb = pp.tile([128, 2], F32, tag="invk")
            nc.sync.dma_start(invk_sb[:], invk[:])
            bk_sb = pp.tile([128, 2], F32, tag="bk")
            nc.sync.dma_start(bk_sb[:], bk[:])
            ones_r = pp.tile([1, 64], F32R, tag="ones")
            nc.vector.memset(ones_r[:].bitcast(F32), 1.0)

            # one-hot row-select matrices for the s-recip broadcast matmul
            # (s rows live at partitions 0/32/64/96 - 32-aligned bases)
            sel4 = []
            for h in range(HPC):
                s4 = pp.tile([97, 64], F32R, tag=f"sel{h}", name=f"sel{h}")
                nc.vector.memset(s4[:].bitcast(F32), 0.0)
                nc.vector.memset(s4[32 * h:32 * h + 1, :].bitcast(F32), 1.0)
                sel4.append(s4)

            # QT/KT assembled feature tiles (f32r), V tiles (f32r, 65-stride)
            qt = [qkp.tile([128, S], F32R, tag=f"qt{h}", name=f"qt{h}") for h in range(HPC)]
            kt = [qkp.tile([128, S], F32R, tag=f"kt{h}", name=f"kt{h}") for h in range(HPC)]
            vt = [vp.tile([128, HPC * 65], F32R, tag=f"v{s}", name=f"v{s}") for s in range(NS)]

            # ---- phase 1: transpose passes + features + V ----
            with (
                tc.tile_pool(name="xT_lo", bufs=1) as xlo,
                tc.tile_pool(name="ph1", bufs=1) as ph1,
            ):
                xT = [None] * 8
                for od in range(2):
                    xT[od] = xlo.tile([128, S], F32R, tag=f"xT{od}", name=f"xT{od}")

                with (
                    tc.tile_pool(name="xT_hi", bufs=1) as xhi,
                    tc.tile_pool(name="xstage", bufs=2) as xs_pool,
                    tc.tile_pool(name="vwst", bufs=2) as vwst,
                    tc.tile_pool(name="tr_ps", bufs=2, space="PSUM") as trp,
                    tc.tile_pool(name="v_ps", bufs=2, space="PSUM") as vps,
                ):
                    for od in range(2, 8):
                        xT[od] = xhi.tile([128, S], F32R, tag=f"xT{od}", name=f"xT{od}")

                    # vwT od tiles -> f32r
                    vwr = []
                    for od in range(8):
                        vwf = vwst.tile([128, CW], F32, tag="vwf", name=f"vwf{od}")
                        nc.sync.dma_start(vwf[:], vwT[od * 128:(od + 1) * 128, :])
                        vwc = ph1.tile([128, CW], F32R, tag=f"vwr{od}", name=f"vwr{od}")
                        nc.vector.tensor_copy(vwc[:], vwf[:])
                        vwr.append(vwc)

                    def transpose_od(si, xs, od):
                        tp = trp.tile([128, 128], F32, tag="tp", name="tp")
                        nc.tensor.transpose(
                            tp[:], xs[:, od * 128:(od + 1) * 128], ident_sb[:]
                        )
                        if od % 2 == 0:
                            nc.vector.tensor_copy(
                                xT[od][:, si * 128:(si + 1) * 128], tp[:]
                            )
                        else:
                            nc.scalar.copy(xT[od][:, si * 128:(si + 1) * 128], tp[:])

                    # pass A: od 0,1 only (unblocks features early)
                    for si in range(NS):
                        xs = xs_pool.tile([128, D], F32, tag="xs", name="xsA")
                        nc.sync.dma_start(xs[:], xb[si * 128:(si + 1) * 128, :])
                        for od in range(2):
                            transpose_od(si, xs, od)

                    # ---- Euler features from xT[0:2] ----
                    FCH = 512  # feature chunk width
                    with tc.tile_pool(name="feat", bufs=1) as fp:
                        for t in range(2):
                            for qk, (inv_sb, b_sb, dstset) in enumerate((
                                (invq_sb, bq_sb, qt),
                                (invk_sb, bk_sb, kt),
                            )):
                                for ch in range(S // FCH):
                                    cs = slice(ch * FCH, (ch + 1) * FCH)
                                    # u = theta*s/4096 (|u| <= ~0.92)
                                    ts_ = fp.tile([128, FCH], F32, tag="tA", bufs=2, name="ts")
                                    nc.vector.tensor_scalar(
                                        ts_[:], xT[t][:, cs],
                                        inv_sb[:, t:t + 1], b_sb[:, t:t + 1],
                                        ALU.mult, ALU.add,
                                    )
                                    # ks = theta*s (ACT affine copy)
                                    ks = fp.tile([128, FCH], F32, tag="tB", bufs=2, name="ks")
                                    nc.scalar.activation(ks[:], ts_[:], AF.Copy,
                                                         scale=4096.0)
                                    # k = round(ks) via i32 cast (RNE on hw)
                                    ki = fp.tile([128, FCH], I32, tag="tC", bufs=2, name="ki")
                                    nc.vector.tensor_copy(ki[:], ks[:])
                                    ji = fp.tile([128, FCH], I32, tag="tD", bufs=2, name="ji")
                                    nc.vector.tensor_copy(ji[:], ts_[:])
                                    kf = fp.tile([128, FCH], F32, tag="tB", bufs=2, name="kf")
                                    nc.vector.tensor_copy(kf[:], ki[:])
                                    jf = fp.tile([128, FCH], F32, tag="tA", bufs=2, name="jf")
                                    nc.vector.tensor_copy(jf[:], ji[:])
                                    kw0 = fp.tile([128, FCH], F32, tag="tE", bufs=2, name="kw0")
                                    nc.vector.scalar_tensor_tensor(
                                        kw0[:], jf[:], -4096.0, kf[:], ALU.mult, ALU.add
                                    )
                                    # clamp to the Sin domain (identity on hw)
                                    kw = fp.tile([128, FCH], F32, tag="tF", bufs=2, name="kw")
                                    nc.vector.tensor_scalar(
                                        kw[:], kw0[:], -2048.0, 2048.0, ALU.max, ALU.min
                                    )
                                    # cos wrap: kwc = kw - 4096*(kw > 1024)
                                    jc = fp.tile([128, FCH], F32, tag="tE", bufs=2, name="jc")
                                    nc.vector.tensor_scalar(
                                        jc[:], kw[:], 1024.0, -4096.0, ALU.is_gt,
                                        ALU.mult
                                    )
                                    kwc = fp.tile([128, FCH], F32, tag="tD", bufs=2, name="kwc")
                                    nc.vector.tensor_tensor(kwc[:], kw[:], jc[:],
                                                            ALU.add)
                                    # 4 Sin ops -> assembled [cos; sin] tiles
                                    for hh in range(2):
                                        dtile = dstset[2 * t + hh]
                                        rows = slice(hh * 64, hh * 64 + 64)
                                        nc.scalar.activation(
                                            dtile[0:64, cs], kwc[rows, :], AF.Sin,
                                            bias=float(np.pi / 2), scale=C_LUT,
                                        )
                                        nc.scalar.activation(
                                            dtile[64:128, cs], kw[rows, :], AF.Sin,
                                            scale=C_LUT,
                                        )

                    # pass B: od 2..7 (re-DMA x)
                    for si in range(NS):
                        xs = xs_pool.tile([128, D], F32, tag="xs", name="xsB")
                        nc.sync.dma_start(xs[:], xb[si * 128:(si + 1) * 128, :])
                        for od in range(2, 8):
                            transpose_od(si, xs, od)

                    # ---- V = x @ vwT (f32r, augmented with ones col) ----
                    for si in range(NS):
                        vpsum = vps.tile([128, CW], F32, tag="vpsum", name="vpsum")
                        for od in range(8):
                            nc.tensor.matmul(
                                vpsum[:],
                                xT[od][:, si * 128:(si + 1) * 128],
                                vwr[od][:],
                                start=(od == 0),
                                stop=(od == 7),
                            )
                        # strided copy into [V_h | 1] blocks of width 65
                        dst = vt[si][:].rearrange("p (h w) -> p h w", w=65)[:, :, 0:64]
                        src = vpsum[:].rearrange("p (h w) -> p h w", w=64)
                        nc.vector.tensor_copy(dst, src)
                        onescol = vt[si][:].rearrange("p (h w) -> p h w", w=65)[
                            :, :, 64:65
                        ]
                        nc.gpsimd.memset(onescol.bitcast(F32), 1.0)

            # owT head-pair tiles (128, D), cast to f32r (staging freed)
            owr = []
            with tc.tile_pool(name="owstage", bufs=2) as ows:
                for hp in range(HPC // 2):
                    owf = ows.tile([128, D], F32, tag="owf", name=f"owf{hp}")
                    nc.sync.dma_start(owf[:], owT[hp * 128:(hp + 1) * 128, :])
                    owc = pp.tile([128, D], F32R, tag=f"owr{hp}", name=f"owr{hp}")
                    nc.vector.tensor_copy(owc[:], owf[:])
                    owr.append(owc)

            # ---- phase 2: attention + projection ----
            with (
                tc.tile_pool(name="attnT", bufs=18) as ap,
                tc.tile_pool(name="osb", bufs=2) as op,
                tc.tile_pool(name="sc_ps", bufs=4, space="PSUM") as scp,
                tc.tile_pool(name="o_ps", bufs=2, space="PSUM") as opp,
                tc.tile_pool(name="bc_ps", bufs=1, space="PSUM") as bcp,
                tc.tile_pool(name="pr_ps", bufs=1, space="PSUM") as prp,
            ):
                inv_scale = float(1.0 / np.float32(SCALE))
                for qw in range(NQW):
                    srow4 = op.tile([97, SQW], F32, tag="srow4", name="srow4", bufs=2)
                    nc.gpsimd.memset(srow4[:], 1.0)
                    oraws = []
                    for h in range(HPC):
                        nkb = 4 * qw + 4
                        ats = []
                        for kb in range(nkb):
                            sc = scp.tile([128, SQW], F32, tag="sc", name="sc")
                            nc.tensor.matmul(
                                sc[:],
                                kt[h][:, kb * KBS:(kb + 1) * KBS],
                                qt[h][:, qw * SQW:(qw + 1) * SQW],
                                start=True, stop=True,
                            )
                            at = ap.tile([128, SQW], F32R, tag="attnT", name="at")
                            nc.scalar.activation(
                                at[:], sc[:], AF.Exp, scale=inv_scale
                            )
                            r = kb - 4 * qw
                            if r >= 0:
                                nc.vector.tensor_tensor(
                                    at[:, r * 128:(r + 1) * 128],
                                    at[:, r * 128:(r + 1) * 128],
                                    tri_sb[:], ALU.mult,
                                )
                                if r > 0:
                                    nc.gpsimd.memset(at[:, 0:r * 128].bitcast(F32), 0.0)
                            ats.append(at)

                        o_ps = opp.tile([65, SQW], F32, tag="o", name="o_ps")
                        for kb in range(nkb):
                            nc.tensor.matmul(
                                o_ps[:],
                                vt[kb][:, h * 65:(h + 1) * 65],
                                ats[kb][:],
                                start=(kb == 0), stop=(kb == nkb - 1),
                            )
                        # stash the raw outT and the s row (row 64)
                        nc.scalar.copy(srow4[32 * h:32 * h + 1, :], o_ps[64:65, :])
                        oraw = op.tile([64, SQW], F32, tag=f"oraw{h}",
                                       name=f"oraw{h}", bufs=2)
                        nc.vector.tensor_copy(oraw[:], o_ps[0:64, :])
                        oraws.append(oraw)

                    # batched reciprocal of the 4 softmax denumerator rows
                    srec4 = op.tile([97, SQW], F32, tag="srec4", name="srec4", bufs=2)
                    nc.vector.reciprocal(srec4[:], srow4[:])
                    srecr4 = op.tile([97, SQW], F32R, tag="srecr4", name="srecr4",
                                     bufs=2)
                    nc.vector.tensor_copy(srecr4[:], srec4[:])

                    # normalize into head-pair tiles (128, SQW)
                    pairs = []
                    for hp in range(HPC // 2):
                        pair = op.tile([128, SQW], F32R, tag=f"pair{hp}",
                                       name=f"pair{hp}", bufs=2)
                        pairs.append(pair)
                    for h in range(HPC):
                        bc = bcp.tile([64, SQW], F32, tag="bc", name="bc")
                        nc.tensor.matmul(
                            bc[:], sel4[h][:], srecr4[:], start=True, stop=True
                        )
                        if h % 2 == 0:
                            nc.vector.tensor_tensor(
                                pairs[h // 2][0:64, :], oraws[h][:], bc[:], ALU.mult
                            )
                        else:
                            tmp = op.tile([64, SQW], F32R, tag="ntmp", name="ntmp",
                                          bufs=2)
                            nc.vector.tensor_tensor(
                                tmp[:], oraws[h][:], bc[:], ALU.mult
                            )
                            nc.scalar.copy(
                                pairs[h // 2][64:128, :], tmp[:]
                            )

                    for od in range(8):
                        pr = prp.tile([128, SQW], F32, tag="pr", name="pr")
                        for hp in range(HPC // 2):
                            nc.tensor.matmul(
                                pr[:],
                                owr[hp][:, od * 128:(od + 1) * 128],
                                pairs[hp][:],
                                start=(hp == 0), stop=(hp == HPC // 2 - 1),
                            )
                        prsb = op.tile([128, SQW], F32, tag="prsb", name="prsb",
                                       bufs=4)
                        nc.vector.tensor_copy(prsb[:], pr[:])
                        nc.sync.dma_start(
                            outT[od * 128:(od + 1) * 128,
                                 qw * SQW:(qw + 1) * SQW],
                            prsb[:],
                        )

    nc.compile()
    return nc


def _prep_inputs(x, w_q, b_q, w_k, b_k, v_w, out_w):
    """Build the 8 per-core input maps (host-side sharding)."""
    s_over = np.float64(LUT) / TWO_PI
    in_maps = []
    tri = np.triu(np.ones((128, 128), dtype=np.float32))  # keep q>=k: g>=p
    ident = np.eye(128, dtype=np.float32)

    wq = w_q.reshape(D)
    bqv = b_q.reshape(D)
    wk = w_k.reshape(D)
    bkv = b_k.reshape(D)

    for c in range(NCORES):
        b = c // 4
        h0 = (c % 4) * HPC
        colbase = h0 * DH
        cols = np.arange(colbase, colbase + CW)
        rest = np.concatenate([np.arange(0, colbase), np.arange(colbase + CW, D)])
        perm = np.concatenate([cols, rest])

        xb = np.ascontiguousarray(x[b][:, perm], dtype=np.float32)
        vwT = np.ascontiguousarray(v_w[cols][:, perm].T, dtype=np.float32)
        owT = np.ascontiguousarray(out_w[:, cols].T, dtype=np.float32)

        def featparams(w, bias):
            inv = (s_over / (1.0 + np.abs(w[cols].astype(np.float64))) / LUT)
            bb = bias[cols].astype(np.float64) * s_over / LUT
            return (inv.reshape(2, 128).T.astype(np.float32).copy(),
                    bb.reshape(2, 128).T.astype(np.float32).copy())

        iq, bq_ = featparams(wq, bqv)
        ik, bk_ = featparams(wk, bkv)

        in_maps.append(dict(
            xb=xb, vwT=vwT, owT=owT,
            invq=iq, bq=bq_, invk=ik, bk=bk_,
            tri=tri, ident=ident,
        ))
    return in_maps


def kernel(x, w_q, b_q, w_k, b_k, v_w, out_w, _trace=False):
    x = np.asarray(x, dtype=np.float32)
    w_q = np.asarray(w_q, dtype=np.float32)
    b_q = np.asarray(b_q, dtype=np.float32)
    w_k = np.asarray(w_k, dtype=np.float32)
    b_k = np.asarray(b_k, dtype=np.float32)
    v_w = np.asarray(v_w, dtype=np.float32)
    out_w = np.asarray(out_w, dtype=np.float32)

    if "nc" not in _CACHE:
        _CACHE["nc"] = _build_nc()
    nc = _CACHE["nc"]

    in_maps = _prep_inputs(x, w_q, b_q, w_k, b_k, v_w, out_w)
    res = run_bass_kernel_spmd(
        nc, in_maps, core_ids=list(range(NCORES)), trace=_trace
    )
    out = np.zeros((B, S, D), dtype=np.float32)
    for c in range(NCORES):
        out[c // 4] += res.results[c]["outT"].T
    if _trace:
        kernel._last_result = res
    return out


# revision 16
# speedup vs baseline: 2.0329x; 1.0384x over previous
"""Trainium2 Bass kernel for nn_EulerCausalAttention_75892072121064.

Sharding: batch*heads across 8 cores (core c -> batch c//4, heads 4*(c%4)..+4).
Each core runs an identical program on column-permuted inputs (its 4 heads'
columns moved to the front), computes transposed-layout causal attention for
its (b, 4-head) slice plus the out-proj partial, and writes outT (D, S).
Host sums the 4 per-batch partials and transposes back.

All big matmuls run in float32r (TF32-like, ~1.6e-4 rel err, full PE rate).
The sin/cos LUT of the reference is reproduced exactly: idx = round(theta *
4096/2pi) (f32->i32 cast = round-to-nearest), wrapped to [-2048, 2048] so the
ACT Sin (accurate on [-pi, pi]) evaluates sin/cos at the exact grid angles.
"""
import sys

import numpy as np

sys.path.insert(0, "/opt/trn_rl_repo")

from concourse import bacc, mybir  # noqa: E402
import concourse.tile as tile  # noqa: E402
from concourse.bass_utils import run_bass_kernel_spmd  # noqa: E402

B, S, D, H, DH = 2, 2048, 1024, 16, 64
LUT = 4096
TWO_PI = 2.0 * np.pi
SCALE = float(np.sqrt(np.float32(2.0 * DH)))  # sqrt(128) as f32
NCORES = 8
HPC = 4            # heads per core
CW = HPC * DH      # 256 cols per core
E = 128            # euler feature dim (cos|sin)
SQW = 512          # q window
NQW = S // SQW
KBS = 128          # k block size
C_LUT = float(np.float32(TWO_PI / LUT))

F32 = mybir.dt.float32
F32R = mybir.dt.float32r
I32 = mybir.dt.int32
AF = mybir.ActivationFunctionType
ALU = mybir.AluOpType

_CACHE = {}


def _build_nc():
    nc = bacc.Bacc("TRN2", debug=False, num_devices=NCORES)
    # const AP for the pi/2 Sin bias
    t = nc.alloc_sbuf_tensor("const-f32-halfpi", [128, 1], F32)
    nc.gpsimd.memset(t.ap(), float(np.pi / 2))
    nc.const_aps.aps[(F32, float(np.pi / 2))] = t.ap()
    nc.all_engine_barrier()

    xb = nc.dram_tensor("xb", [S, D], F32, kind="ExternalInput")
    vwT = nc.dram_tensor("vwT", [D, CW], F32, kind="ExternalInput")
    owT = nc.dram_tensor("owT", [CW, D], F32, kind="ExternalInput")
    invq = nc.dram_tensor("invq", [128, 2], F32, kind="ExternalInput")
    bq = nc.dram_tensor("bq", [128, 2], F32, kind="ExternalInput")
    invk = nc.dram_tensor("invk", [128, 2], F32, kind="ExternalInput")
    bk = nc.dram_tensor("bk", [128, 2], F32, kind="ExternalInput")
    tri = nc.dram_tensor("tri", [128, 128], F32, kind="ExternalInput")
    ident = nc.dram_tensor("ident", [128, 128], F32, kind="ExternalInput")
    outT = nc.dram_tensor("outT", [D, S], F32, kind="ExternalOutput")

    NS = S // 128  # number of 128-row seq tiles

    with tile.TileContext(nc) as tc:
        with (
            tc.tile_pool(name="persist", bufs=1) as pp,
            tc.tile_pool(name="qkt", bufs=1) as qkp,
            tc.tile_pool(name="vtiles", bufs=1) as vp,
        ):
            # ---- small constants ----
            ident_sb = pp.tile([128, 128], F32, tag="ident")
            nc.sync.dma_start(ident_sb[:], ident[:])
            tri_sb = pp.tile([128, 128], F32, tag="tri")
            nc.sync.dma_start(tri_sb[:], tri[:])
            invq_sb = pp.tile([128, 2], F32, tag="invq")
            nc.sync.dma_start(invq_sb[:], invq[:])
            bq_sb = pp.tile([128, 2], F32, tag="bq")
            nc.sync.dma_start(bq_sb[:], bq[:])
            invk_sb = pp.tile([128, 2], F32, tag="invk")
            nc.sync.dma_start(invk_sb[:], invk[:])
            bk_sb = pp.tile([128, 2], F32, tag="bk")
            nc.sync.dma_start(bk_sb[:], bk[:])
            ones_r = pp.tile([1, 64], F32R, tag="ones")
            nc.vector.memset(ones_r[:].bitcast(F32), 1.0)

            # one-hot row-select matrices for the s-recip broadcast matmul
            # (s rows live at partitions 0/32/64/96 - 32-aligned bases)
            sel4 = []
            for h in range(HPC):
                s4 = pp.tile([97, 64], F32R, tag=f"sel{h}", name=f"sel{h}")
                nc.vector.memset(s4[:].bitcast(F32), 0.0)
                nc.vector.memset(s4[32 * h:32 * h + 1, :].bitcast(F32), 1.0)
                sel4.append(s4)

            # QT/KT assembled feature tiles (f32r), V tiles (f32r, 65-stride)
            qt = [qkp.tile([128, S], F32R, tag=f"qt{h}", name=f"qt{h}") for h in range(HPC)]
            kt = [qkp.tile([128, S], F32R, tag=f"kt{h}", name=f"kt{h}") for h in range(HPC)]
            vt = [vp.tile([128, HPC * 65], F32R, tag=f"v{s}", name=f"v{s}") for s in range(NS)]

            # ---- phase 1: transpose passes + features + V ----
            with (
                tc.tile_pool(name="xT_lo", bufs=1) as xlo,
                tc.tile_pool(name="ph1", bufs=1) as ph1,
            ):
                xT = [None] * 8
                for od in range(2):
                    xT[od] = xlo.tile([128, S], F32R, tag=f"xT{od}", name=f"xT{od}")

                with (
                    tc.tile_pool(name="xT_hi", bufs=1) as xhi,
                    tc.tile_pool(name="xstage", bufs=2) as xs_pool,
                    tc.tile_pool(name="vwst", bufs=2) as vwst,
                    tc.tile_pool(name="tr_ps", bufs=2, space="PSUM") as trp,
                    tc.tile_pool(name="v_ps", bufs=2, space="PSUM") as vps,
                ):
                    for od in range(2, 8):
                        xT[od] = xhi.tile([128, S], F32R, tag=f"xT{od}", name=f"xT{od}")

                    # vwT od tiles -> f32r
                    vwr = []
                    for od in range(8):
                        vwf = vwst.tile([128, CW], F32, tag="vwf", name=f"vwf{od}")
                        nc.sync.dma_start(vwf[:], vwT[od * 128:(od + 1) * 128, :])
                        vwc = ph1.tile([128, CW], F32R, tag=f"vwr{od}", name=f"vwr{od}")
                        nc.vector.tensor_copy(vwc[:], vwf[:])
                        vwr.append(vwc)

                    def transpose_pass(ods, pfx):
                        # groups of 4 seq-tiles -> one (128,512) psum -> 1 copy
                        for sg in range(NS // 4):
                            xs4 = []
                            for j in range(4):
                                si = sg * 4 + j
                                xs = xs_pool.tile([128, D], F32, tag="xs",
                                                  name=f"xs{pfx}{si}", bufs=4)
                                nc.sync.dma_start(
                                    xs[:], xb[si * 128:(si + 1) * 128, :])
                                xs4.append(xs)
                            for od in ods:
                                tp = trp.tile([128, 512], F32, tag="tp", name="tp")
                                for j in range(4):
                                    nc.tensor.transpose(
                                        tp[:, j * 128:(j + 1) * 128],
                                        xs4[j][:, od * 128:(od + 1) * 128],
                                        ident_sb[:],
                                    )
                                dst = xT[od][:, sg * 512:(sg + 1) * 512]
                                if od % 2 == 0:
                                    nc.vector.tensor_copy(dst, tp[:])
                                else:
                                    nc.scalar.copy(dst, tp[:])

                    # pass A: od 0,1 only (unblocks features early)
                    transpose_pass(range(2), "A")

                    # ---- Euler features from xT[0:2] ----
                    FCH = 512  # feature chunk width
                    with tc.tile_pool(name="feat", bufs=1) as fp:
                        for t in range(2):
                            for qk, (inv_sb, b_sb, dstset) in enumerate((
                                (invq_sb, bq_sb, qt),
                                (invk_sb, bk_sb, kt),
                            )):
                                for ch in range(S // FCH):
                                    cs = slice(ch * FCH, (ch + 1) * FCH)
                                    # u = theta*s/4096 (|u| <= ~0.92)
                                    ts_ = fp.tile([128, FCH], F32, tag="tA", bufs=2, name="ts")
                                    nc.vector.tensor_scalar(
                                        ts_[:], xT[t][:, cs],
                                        inv_sb[:, t:t + 1], b_sb[:, t:t + 1],
                                        ALU.mult, ALU.add,
                                    )
                                    # ks = theta*s (ACT affine copy)
                                    ks = fp.tile([128, FCH], F32, tag="tB", bufs=2, name="ks")
                                    nc.scalar.activation(ks[:], ts_[:], AF.Copy,
                                                         scale=4096.0)
                                    # k = round(ks) via i32 cast (RNE on hw)
                                    ki = fp.tile([128, FCH], I32, tag="tC", bufs=2, name="ki")
                                    nc.vector.tensor_copy(ki[:], ks[:])
                                    ji = fp.tile([128, FCH], I32, tag="tD", bufs=2, name="ji")
                                    nc.vector.tensor_copy(ji[:], ts_[:])
                                    kf = fp.tile([128, FCH], F32, tag="tB", bufs=2, name="kf")
                                    nc.vector.tensor_copy(kf[:], ki[:])
                                    jf = fp.tile([128, FCH], F32, tag="tA", bufs=2, name="jf")
                                    nc.vector.tensor_copy(jf[:], ji[:])
                                    kw0 = fp.tile([128, FCH], F32, tag="tE", bufs=2, name="kw0")
                                    nc.vector.scalar_tensor_tensor(
                                        kw0[:], jf[:], -4096.0, kf[:], ALU.mult, ALU.add
                                    )
                                    # clamp to the Sin domain (identity on hw)
                                    kw = fp.tile([128, FCH], F32, tag="tF", bufs=2, name="kw")
                                    nc.vector.tensor_scalar(
                                        kw[:], kw0[:], -2048.0, 2048.0, ALU.max, ALU.min
                                    )
                                    # cos wrap: kwc = kw - 4096*(kw > 1024)
                                    jc = fp.tile([128, FCH], F32, tag="tE", bufs=2, name="jc")
                                    nc.vector.tensor_scalar(
                                        jc[:], kw[:], 1024.0, -4096.0, ALU.is_gt,
                                        ALU.mult
                                    )
                                    kwc = fp.tile([128, FCH], F32, tag="tD", bufs=2, name="kwc")
                                    nc.vector.tensor_tensor(kwc[:], kw[:], jc[:],
                                                            ALU.add)
                                    # 4 Sin ops -> assembled [cos; sin] tiles
                                    for hh in range(2):
                                        dtile = dstset[2 * t + hh]
                                        rows = slice(hh * 64, hh * 64 + 64)
                                        nc.scalar.activation(
                                            dtile[0:64, cs], kwc[rows, :], AF.Sin,
                                            bias=float(np.pi / 2), scale=C_LUT,
                                        )
                                        nc.scalar.activation(
                                            dtile[64:128, cs], kw[rows, :], AF.Sin,
                                            scale=C_LUT,
                                        )

                    # pass B: od 2..7 (re-DMA x)
                    transpose_pass(range(2, 8), "B")

                    # ---- V = x @ vwT (f32r, augmented with ones col) ----
                    for si in range(NS):
                        vpsum = vps.tile([128, CW], F32, tag="vpsum", name="vpsum")
                        for od in range(8):
                            nc.tensor.matmul(
                                vpsum[:],
                                xT[od][:, si * 128:(si + 1) * 128],
                                vwr[od][:],
                                start=(od == 0),
                                stop=(od == 7),
                            )
                        # strided copy into [V_h | 1] blocks of width 65
                        dst = vt[si][:].rearrange("p (h w) -> p h w", w=65)[:, :, 0:64]
                        src = vpsum[:].rearrange("p (h w) -> p h w", w=64)
                        nc.vector.tensor_copy(dst, src)
                        onescol = vt[si][:].rearrange("p (h w) -> p h w", w=65)[
                            :, :, 64:65
                        ]
                        nc.gpsimd.memset(onescol.bitcast(F32), 1.0)

            # owT head-pair tiles (128, D), cast to f32r (staging freed)
            owr = []
            with tc.tile_pool(name="owstage", bufs=2) as ows:
                for hp in range(HPC // 2):
                    owf = ows.tile([128, D], F32, tag="owf", name=f"owf{hp}")
                    nc.sync.dma_start(owf[:], owT[hp * 128:(hp + 1) * 128, :])
                    owc = pp.tile([128, D], F32R, tag=f"owr{hp}", name=f"owr{hp}")
                    nc.vector.tensor_copy(owc[:], owf[:])
                    owr.append(owc)

            # ---- phase 2: attention + projection ----
            with (
                tc.tile_pool(name="attnT", bufs=18) as ap,
                tc.tile_pool(name="osb", bufs=2) as op,
                tc.tile_pool(name="sc_ps", bufs=4, space="PSUM") as scp,
                tc.tile_pool(name="o_ps", bufs=2, space="PSUM") as opp,
                tc.tile_pool(name="bc_ps", bufs=1, space="PSUM") as bcp,
                tc.tile_pool(name="pr_ps", bufs=1, space="PSUM") as prp,
            ):
                inv_scale = float(1.0 / np.float32(SCALE))
                for qw in range(NQW):
                    srow4 = op.tile([97, SQW], F32, tag="srow4", name="srow4", bufs=2)
                    nc.gpsimd.memset(srow4[:], 1.0)
                    oraws = []
                    for h in range(HPC):
                        nkb = 4 * qw + 4
                        ats = []
                        for kb in range(nkb):
                            sc = scp.tile([128, SQW], F32, tag="sc", name="sc")
                            nc.tensor.matmul(
                                sc[:],
                                kt[h][:, kb * KBS:(kb + 1) * KBS],
                                qt[h][:, qw * SQW:(qw + 1) * SQW],
                                start=True, stop=True,
                            )
                            at = ap.tile([128, SQW], F32R, tag="attnT", name="at")
                            nc.scalar.activation(
                                at[:], sc[:], AF.Exp, scale=inv_scale
                            )
                            r = kb - 4 * qw
                            if r >= 0:
                                nc.vector.tensor_tensor(
                                    at[:, r * 128:(r + 1) * 128],
                                    at[:, r * 128:(r + 1) * 128],
                                    tri_sb[:], ALU.mult,
                                )
                                if r > 0:
                                    nc.gpsimd.memset(at[:, 0:r * 128].bitcast(F32), 0.0)
                            ats.append(at)

                        o_ps = opp.tile([65, SQW], F32, tag="o", name="o_ps")
                        for kb in range(nkb):
                            nc.tensor.matmul(
                                o_ps[:],
                                vt[kb][:, h * 65:(h + 1) * 65],
                                ats[kb][:],
                                start=(kb == 0), stop=(kb == nkb - 1),
                            )
                        # stash the raw outT and the s row (row 64)
                        nc.scalar.copy(srow4[32 * h:32 * h + 1, :], o_ps[64:65, :])
                        oraw = op.tile([64, SQW], F32, tag=f"oraw{h}",
                                       name=f"oraw{h}", bufs=2)
                        nc.vector.tensor_copy(oraw[:], o_ps[0:64, :])
                        oraws.append(oraw)

                    # batched reciprocal of the 4 softmax denumerator rows
                    srec4 = op.tile([97, SQW], F32, tag="srec4", name="srec4", bufs=2)
                    nc.vector.reciprocal(srec4[:], srow4[:])
                    srecr4 = op.tile([97, SQW], F32R, tag="srecr4", name="srecr4",
                                     bufs=2)
                    nc.vector.tensor_copy(srecr4[:], srec4[:])

                    # normalize into head-pair tiles (128, SQW)
                    pairs = []
                    for hp in range(HPC // 2):
                        pair = op.tile([128, SQW], F32R, tag=f"pair{hp}",
                                       name=f"pair{hp}", bufs=2)
                        pairs.append(pair)
                    for h in range(HPC):
                        bc = bcp.tile([64, SQW], F32, tag="bc", name="bc")
                        nc.tensor.matmul(
                            bc[:], sel4[h][:], srecr4[:], start=True, stop=True
                        )
                        if h % 2 == 0:
                            nc.vector.tensor_tensor(
                                pairs[h // 2][0:64, :], oraws[h][:], bc[:], ALU.mult
                            )
                        else:
                            tmp = op.tile([64, SQW], F32R, tag="ntmp", name="ntmp",
                                          bufs=2)
                            nc.vector.tensor_tensor(
                                tmp[:], oraws[h][:], bc[:], ALU.mult
                            )
                            nc.scalar.copy(
                                pairs[h // 2][64:128, :], tmp[:]
                            )

                    for od in range(8):
                        pr = prp.tile([128, SQW], F32, tag="pr", name="pr")
                        for hp in range(HPC // 2):
                            nc.tensor.matmul(
                                pr[:],
                                owr[hp][:, od * 128:(od + 1) * 128],
                                pairs[hp][:],
                                start=(hp == 0), stop=(hp == HPC // 2 - 1),
                            )
                        prsb = op.tile([128, SQW], F32, tag="prsb", name="prsb",
                                       bufs=4)
                        nc.vector.tensor_copy(prsb[:], pr[:])
                        nc.sync.dma_start(
                            outT[od * 128:(od + 1) * 128,
                                 qw * SQW:(qw + 1) * SQW],
                            prsb[:],
                        )

    nc.compile()
    return nc


def _prep_inputs(x, w_q, b_q, w_k, b_k, v_w, out_w):
    """Build the 8 per-core input maps (host-side sharding)."""
    s_over = np.float64(LUT) / TWO_PI
    in_maps = []
    tri = np.triu(np.ones((128, 128), dtype=np.float32))  # keep q>=k: g>=p
    ident = np.eye(128, dtype=np.float32)

    wq = w_q.reshape(D)
    bqv = b_q.reshape(D)
    wk = w_k.reshape(D)
    bkv = b_k.reshape(D)

    for c in range(NCORES):
        b = c // 4
        h0 = (c % 4) * HPC
        colbase = h0 * DH
        cols = np.arange(colbase, colbase + CW)
        rest = np.concatenate([np.arange(0, colbase), np.arange(colbase + CW, D)])
        perm = np.concatenate([cols, rest])

        xb = np.ascontiguousarray(x[b][:, perm], dtype=np.float32)
        vwT = np.ascontiguousarray(v_w[cols][:, perm].T, dtype=np.float32)
        owT = np.ascontiguousarray(out_w[:, cols].T, dtype=np.float32)

        def featparams(w, bias):
            inv = (s_over / (1.0 + np.abs(w[cols].astype(np.float64))) / LUT)
            bb = bias[cols].astype(np.float64) * s_over / LUT
            return (inv.reshape(2, 128).T.astype(np.float32).copy(),
                    bb.reshape(2, 128).T.astype(np.float32).copy())

        iq, bq_ = featparams(wq, bqv)
        ik, bk_ = featparams(wk, bkv)

        in_maps.append(dict(
            xb=xb, vwT=vwT, owT=owT,
            invq=iq, bq=bq_, invk=ik, bk=bk_,
            tri=tri, ident=ident,
        ))
    return in_maps


def kernel(x, w_q, b_q, w_k, b_k, v_w, out_w, _trace=False):
    x = np.asarray(x, dtype=np.float32)
    w_q = np.asarray(w_q, dtype=np.float32)
    b_q = np.asarray(b_q, dtype=np.float32)
    w_k = np.asarray(w_k, dtype=np.float32)
    b_k = np.asarray(b_k, dtype=np.float32)
    v_w = np.asarray(v_w, dtype=np.float32)
    out_w = np.asarray(out_w, dtype=np.float32)

    if "nc" not in _CACHE:
        _CACHE["nc"] = _build_nc()
    nc = _CACHE["nc"]

    in_maps = _prep_inputs(x, w_q, b_q, w_k, b_k, v_w, out_w)
    res = run_bass_kernel_spmd(
        nc, in_maps, core_ids=list(range(NCORES)), trace=_trace
    )
    out = np.zeros((B, S, D), dtype=np.float32)
    for c in range(NCORES):
        out[c // 4] += res.results[c]["outT"].T
    if _trace:
        kernel._last_result = res
    return out
